# revision 11
# baseline (speedup 1.0000x reference)
"""Trainium2 Bass kernel for nn_CustomLoss_62079457296845.

Computes L = mean((y_hat - y)^2) + mean((y_hat - mag4c)^2) where
y_hat = (mag4uc - rowdot(A, beta + c) - y_mean) / y_scale, over
N=4194304 rows, D=18 features.

Strategy: pure data parallel over 8 NeuronCores; each core streams its
524288-row shard through SBUF as 8 tiles of [128 x 512 x 18] in bf16
(host-side cast; harness gate is rel_err < 2e-2, bf16 keeps it ~1e-3).
Per tile:
  ACT:  prefill c-tile with the replicated beta pattern (Copy)
  DMA:  SWDGE accumulate-add streams HBM c on top (c + beta, in-flight)
  DVE:  prod = A * (c+beta)  (bf16 tensor_tensor, 2x mode)
        rd   = reduce_sum(prod, axis=-1)
        h    = u - rd ; t1 = s*h - y ; t2 = s*h - m
  ACT:  Square activation with bias=-y_mean*s, accum_out -> per-tile sums
Per-core output [128, 2*NT] f32 partial sums; host sums in f64.
"""

import os
import sys

import numpy as np

for _p in ("/opt/trn_rl_repo",):
    if _p not in sys.path and os.path.isdir(_p):
        sys.path.insert(0, _p)

N = 4194304
D = 18
NCORES = 8
R = N // NCORES          # rows per core
P = 128                  # SBUF partitions
W = 512                  # rows per partition per tile
NT = R // (P * W)        # tiles per core (8)


def _to_bf16(x):
    """Fast numpy f32 -> bf16 cast (round-to-nearest-even)."""
    import ml_dtypes

    x = np.ascontiguousarray(np.asarray(x, np.float32))
    u = x.view(np.uint32)
    v = u + 0x7FFF + ((u >> 16) & 1)
    return (v >> 16).astype(np.uint16).view(ml_dtypes.bfloat16)


_MUL_SCAN = None


def _register_mul_scan():
    """Register a custom fused DVE op: continuous prefix-sum of Src0*Src1.

    out[p, k] = sum_{j<=k} in0[p, j] * in1[p, j]  (fp32 internal, whole
    free-dim stream).  Row dots are recovered by sampling every D-th
    element and differencing adjacent samples.
    """
    global _MUL_SCAN
    if _MUL_SCAN is not None:
        return _MUL_SCAN
    import concourse.dve_ops as dve_ops
    from concourse.dve_spec import Spec, Src0, Src1, scan, AluOp, lower
    from concourse.dve_uop import DveOpSpec

    name = "MUL_SCAN_ANT"
    if name in dve_ops._SUB_OPCODE_FOR_NAME:
        _MUL_SCAN = next(o for o in dve_ops.OPS if o.name == name)
        return _MUL_SCAN

    def _ref(in0, in1, s0, s1, imm2):
        p = in0.shape[0]
        prod = (np.asarray(in0, np.float32).reshape(p, -1)
                * np.asarray(in1, np.float32).reshape(p, -1))
        return np.cumsum(prod, axis=-1, dtype=np.float32).reshape(in0.shape)

    spec = Spec(body=scan(AluOp.ADD, Src0 * Src1), reference=_ref)
    row = dve_ops._CUSTOM_DVE_ROW_BASE + len(dve_ops.OPS)
    dve_ops._SUB_OPCODE_FOR_NAME[name] = row
    shas = {}
    for ver in ("v3", "v4"):
        try:
            tmp = DveOpSpec(name=name, opcode=row,
                            uops=lower(spec, ver=ver), rd1_en=True)
            shas[ver] = tmp.sha(ver)
        except Exception:
            pass
    op = dve_ops.DveOp(name, spec, subdim=False, uops_sha=shas)
    dve_ops.OPS.append(op)
    dve_ops.CUSTOM_DVE_SPECS[name] = spec
    _MUL_SCAN = op
    return op


def _build(s: float, bg: float, nt: int = NT, reps: int = 1, w: int = W,
           beta_cce: bool = True, dve_add: bool = False,
           pool_add_tiles: tuple = (), a_bufs: int = 2, c_bufs: int = 3,
           prod_bufs: int = 2, small_bufs: int = 2, sep_prod: bool = True,
           cce_wc: int = 64, fuse: bool = True, pool_tail: bool = True):
    """Build the Bass program. s = 1/y_scale, bg = -y_mean/y_scale."""
    from contextlib import ExitStack

    import concourse.tile as tile
    from concourse import bacc, mybir

    f32 = mybir.dt.float32
    bf16 = mybir.dt.bfloat16
    Alu = mybir.AluOpType

    nc = bacc.Bacc("TRN2", debug=False, target_bir_lowering=False,
                   num_devices=NCORES)

    A_d = nc.dram_tensor("A_t", [nt, P, w, D], bf16, kind="ExternalInput").ap()
    C_d = nc.dram_tensor("C_t", [nt, P, w, D], bf16, kind="ExternalInput").ap()
    YUM_d = nc.dram_tensor("YUM_t", [nt, P, 3, w], bf16,
                           kind="ExternalInput").ap()
    B_d = nc.dram_tensor("B_rep", [1, cce_wc * D], bf16,
                         kind="ExternalInput").ap()
    out_d = nc.dram_tensor("out", [P, 2 * nt], f32, kind="ExternalOutput").ap()

    with ExitStack() as ctx:
        tc = ctx.enter_context(tile.TileContext(nc))
        consts = ctx.enter_context(tc.tile_pool(name="consts", bufs=1))
        apool = ctx.enter_context(tc.tile_pool(name="apool", bufs=a_bufs))
        cpool = ctx.enter_context(tc.tile_pool(name="cpool", bufs=c_bufs))
        ppool = ctx.enter_context(
            tc.tile_pool(name="ppool", bufs=prod_bufs)) if sep_prod else None
        small = ctx.enter_context(tc.tile_pool(name="small", bufs=small_bufs))

        beta_sb = consts.tile([P, cce_wc, D], bf16)
        nc.sync.dma_start(out=beta_sb, in_=B_d.to_broadcast((P, cce_wc * D)))
        _bap = beta_sb[:]
        import concourse.bass as bass
        beta_rep_ap = bass.AP(tensor=_bap.tensor, offset=_bap.offset,
                              ap=[_bap.ap[0], [0, w // cce_wc]]
                              + list(_bap.ap[1:]))

        bias_sb = consts.tile([P, 1], f32)
        nc.vector.memset(bias_sb, float(bg))

        outs = consts.tile([P, 2 * nt], f32)

        for rep in range(reps):
          for i in range(nt):
              a = apool.tile([P, w, D], bf16, tag="a")
              nc.sync.dma_start(out=a, in_=A_d[i])
              c = cpool.tile([P, w, D], bf16, tag="c")
              if beta_cce:
                  # pre-fill with beta pattern, then accumulate the HBM c
                  # tile into it during the DMA (SWDGE CCE add)
                  nc.scalar.activation(out=c, in_=beta_rep_ap,
                                       func=mybir.ActivationFunctionType.Copy)
                  # CCE accumulate: per-descriptor element limit is 2048, so
                  # split the c DMA into chunks of cce_wc rows (cce_wc*D
                  # elems per partition line)
                  for j in range(0, w, cce_wc):
                      nc.gpsimd.dma_start(out=c[:, j:j + cce_wc],
                                          in_=C_d[i, :, j:j + cce_wc],
                                          accum_op=Alu.add)
              else:
                  nc.sync.dma_start(out=c, in_=C_d[i])
                  use_pool = (i % nt) in pool_add_tiles and not dve_add
                  eng = nc.gpsimd if use_pool else nc.vector
                  eng.tensor_tensor(out=c, in0=c, in1=beta_sb, op=Alu.add)
              yum = small.tile([P, 3, w], bf16, tag="yum")
              nc.sync.dma_start(out=yum, in_=YUM_d[i])

              rd = small.tile([P, w], f32, tag="rd")
              tail = nc.gpsimd if pool_tail else nc.vector
              if fuse:
                  op = _register_mul_scan()
                  sc = ppool.tile([P, w, D], f32, tag="sc")
                  nc.vector._custom_dve(op, out=sc, in0=a, in1=c)
                  e = small.tile([P, w + 1], f32, tag="e")
                  tail.memset(e[:, 0:1], 0.0)
                  tail.tensor_copy(out=e[:, 1:], in_=sc[:, :, D - 1])
                  tail.tensor_tensor(out=rd, in0=e[:, 1:], in1=e[:, 0:w],
                                     op=Alu.subtract)
              else:
                  if sep_prod:
                      prod = ppool.tile([P, w, D], bf16, tag="prod")
                  else:
                      prod = c
                  nc.vector.tensor_tensor(out=prod, in0=a, in1=c, op=Alu.mult)
                  nc.vector.tensor_reduce(out=rd, in_=prod,
                                          axis=mybir.AxisListType.X, op=Alu.add)
              h = rd
              tail.scalar_tensor_tensor(out=h, in0=rd, scalar=-1.0,
                                        in1=yum[:, 1], op0=Alu.mult,
                                        op1=Alu.add)
              t1 = small.tile([P, w], f32, tag="t1")
              tail.scalar_tensor_tensor(out=t1, in0=h, scalar=float(s),
                                        in1=yum[:, 0], op0=Alu.mult,
                                        op1=Alu.subtract)
              t2 = small.tile([P, w], f32, tag="t2")
              tail.scalar_tensor_tensor(out=t2, in0=h, scalar=float(s),
                                        in1=yum[:, 2], op0=Alu.mult,
                                        op1=Alu.subtract)
              nc.scalar.activation(out=t1, in_=t1,
                                   func=mybir.ActivationFunctionType.Square,
                                   bias=bias_sb[:], scale=1.0,
                                   accum_out=outs[:, 2 * i:2 * i + 1])
              nc.scalar.activation(out=t2, in_=t2,
                                   func=mybir.ActivationFunctionType.Square,
                                   bias=bias_sb[:], scale=1.0,
                                   accum_out=outs[:, 2 * i + 1:2 * i + 2])

        nc.sync.dma_start(out=out_d, in_=outs)

    nc.compile()
    return nc


def _shard_inputs(c, y, A, mag4uc, mag4c, beta):
    beta_rep = _to_bf16(
        np.tile(np.asarray(beta, np.float32).reshape(D), 64).reshape(1, 64 * D))
    A = np.asarray(A, np.float32)
    c = np.asarray(c, np.float32)
    y = np.asarray(y, np.float32).reshape(N)
    u = np.asarray(mag4uc, np.float32).reshape(N)
    m = np.asarray(mag4c, np.float32).reshape(N)
    in_maps = []
    for k in range(NCORES):
        lo, hi = k * R, (k + 1) * R
        yum = np.stack([y[lo:hi].reshape(NT, P, W),
                        u[lo:hi].reshape(NT, P, W),
                        m[lo:hi].reshape(NT, P, W)], axis=2)
        in_maps.append({
            "A_t": _to_bf16(A[lo:hi]).reshape(NT, P, W, D),
            "C_t": _to_bf16(c[lo:hi]).reshape(NT, P, W, D),
            "YUM_t": _to_bf16(yum),
            "B_rep": beta_rep,
        })
    return in_maps


def _run(inputs: dict, trace: bool = False):
    from concourse.bass_utils import run_bass_kernel_spmd

    y_scale = float(np.asarray(inputs["y_scale"]).reshape(-1)[0])
    y_mean = float(np.asarray(inputs["y_mean"]).reshape(-1)[0])
    s = 1.0 / y_scale
    bg = -y_mean * s

    variant = os.environ.get("KERNEL_VARIANT", "cce")
    nc = _build(s, bg, beta_cce=(variant == "cce"),
                dve_add=(variant == "dveadd"),
                fuse=(variant != "nofuse"))
    in_maps = _shard_inputs(inputs["c"], inputs["y"], inputs["A"],
                            inputs["mag4uc"], inputs["mag4c"], inputs["beta"])
    res = run_bass_kernel_spmd(nc, in_maps, list(range(NCORES)), trace=trace)
    total = np.float64(0.0)
    for r in res.results:
        total += r["out"].astype(np.float64).sum()
    loss = np.float32(total / N)
    return np.asarray(loss, dtype=np.float32), res


def kernel(**inputs) -> np.ndarray:
    out, _ = _run(inputs, trace=False)
    return out
